# revision 19
# baseline (speedup 1.0000x reference)
"""Trainium2 Bass kernel for the variants-attention module.

Model (reference):
    q = (x @ Wq)                          [B,N,H,D]
    kv = variants @ Wkv -> k,v            [V,B,N,H,D] each
    attn = softmax(q.k / sqrt(D)) over V  (per-token attention over variants)
    out = (attn.v) @ Wp + bp              [B,N,C]

Strategy: data-parallel over the B*N = 16384 tokens across 8 NeuronCores
(2048 tokens/core), weights replicated.  Host pre-casts inputs to bf16 and
pre-transposes activations to feature-major so the kernel streams them into
the PE array without on-chip transposes.  All projections run on the tensor
engine in bf16 (fp32 PSUM accumulate).  The attention softmax scale is
folded into Wq on the host (exact: 1/8 is a power of two), and the output
bias bp is added on the host after gathering, so the PE does nothing but
the three projections.  PSUM->SBUF evacuation runs on the scalar (ACT)
engine; the per-token attention over V=4 variants runs on the vector
engine with all tensor_tensor ops in 2x mode (the softmax weights are
written duplicated-in-pairs so the d-broadcast multiply still reads packed
16-bit pairs).  The attended output is transposed back via SBUF->SBUF
xbar DMA-transpose and projected through Wp with a 3-chunk pipeline lag so
the vector-engine chain and transpose DMAs never stall the PE.  Input
tiles stream in on the gpsimd SWDGE ring so the SP HWDGE ring carries
only transposes and output stores.
"""

import numpy as np
import ml_dtypes

import concourse.bass as bass
import concourse.bacc as bacc
import concourse.tile as tile
from concourse import mybir
from concourse.bass_utils import run_bass_kernel_spmd

# ---------------------------------------------------------------------------

V, B, N, C, H = 4, 4, 4096, 768, 12
D = C // H
SCALE = D**-0.5
TOK = B * N
N_CORES = 8
TPC = TOK // N_CORES  # tokens per core

BF16 = mybir.dt.bfloat16
F32 = mybir.dt.float32
CK = C // 128  # 6 feature chunks

nbf16 = ml_dtypes.bfloat16


def build_nc(tpc=TPC, tile_tok=512, repeat=1, loop=1, ablate=None, lag=3):
    """Build the per-core Bass program for `tpc` tokens.

    repeat>1 re-runs the whole computation that many times unrolled;
    loop>1 wraps the body in a hardware For_i loop.  Both are idempotent
    (same outputs) and exist only for timing: with loop~1000 the NEFF's
    execution time dominates the axon dispatch jitter, so wall/loop ~= exec.
    """
    assert tpc % tile_tok == 0 and tile_tok % 128 == 0
    n_tiles = tpc // tile_tok
    n_ch = tile_tok // 128  # 128-token chunks per tile

    nc = bacc.Bacc("TRN2", target_bir_lowering=False, debug=False, num_devices=N_CORES)

    xT = nc.dram_tensor("xT", [C, tpc], BF16, kind="ExternalInput").ap()
    pT = nc.dram_tensor("pT", [V, C, tpc], BF16, kind="ExternalInput").ap()
    wq = nc.dram_tensor("wq", [C, C], BF16, kind="ExternalInput").ap()
    wkv = nc.dram_tensor("wkv", [C, 2 * C], BF16, kind="ExternalInput").ap()
    wp = nc.dram_tensor("wp", [C, C], BF16, kind="ExternalInput").ap()
    out = nc.dram_tensor("out", [tpc, C], BF16, kind="ExternalOutput").ap()

    xT_v = xT.rearrange("(ck p) t -> p ck t", p=128)
    pT_v = pT.rearrange("v (ck p) t -> p v ck t", p=128)

    with tile.TileContext(nc) as tc:
        with (
            tc.tile_pool(name="const", bufs=1) as constp,
            tc.tile_pool(name="xin", bufs=2) as xin,
            tc.tile_pool(name="pin", bufs=2) as pin,
            tc.tile_pool(name="qkv", bufs=2) as qkvp,
            tc.tile_pool(name="attn", bufs=2) as attp,
            tc.tile_pool(name="attT", bufs=4) as attTp,
            tc.tile_pool(name="outs", bufs=2) as outp,
            tc.tile_pool(name="pskv", bufs=2, space="PSUM") as pskv,
            tc.tile_pool(
                name="psqo",
                bufs=4 if (ablate or "").startswith("mmbench") else 1,
                space="PSUM",
            ) as psqo,
        ):
            # --- persistent constants ---
            # first tile's activations load before the big weight tensors so
            # the PE can start as soon as wq + tile0 land.
            xt0 = xin.tile([128, CK, tile_tok], BF16, tag="xt")
            nc.gpsimd.dma_start(xt0[:], xT_v[:, :, 0:tile_tok])
            skip_pt = ablate == "qonly" or (ablate or "").startswith("mmbench")
            pt0 = None
            if not skip_pt:
                pt0 = pin.tile([128, V, CK, tile_tok], BF16, tag="pt")
                for v in range(V):
                    nc.gpsimd.dma_start(pt0[:, v], pT_v[:, v, :, 0:tile_tok])

            wq_sb = constp.tile([128, CK, C], BF16, tag="wq")
            nc.sync.dma_start(wq_sb[:], wq.rearrange("(ck p) o -> p ck o", p=128))
            wkv_sb = constp.tile([128, CK, 2 * C], BF16, tag="wkv")
            nc.sync.dma_start(wkv_sb[:], wkv.rearrange("(ck p) o -> p ck o", p=128))
            wp_sb = constp.tile([128, CK, C], BF16, tag="wp")
            nc.sync.dma_start(wp_sb[:], wp.rearrange("(ck p) o -> p ck o", p=128))

            n_chunks = n_tiles * n_ch

            def emit_q(xt, tc_i):
                """q projection for one 128-token chunk -> SBUF bf16."""
                ts = slice(tc_i * 128, (tc_i + 1) * 128)
                q_ps = psqo.tile([128, C], F32, tag="qo")
                for ck in range(CK):
                    lhsT = xt[:, ck, ts]
                    nc.tensor.matmul(
                        q_ps[:, 0:512], lhsT, wq_sb[:, ck, 0:512],
                        start=(ck == 0), stop=(ck == CK - 1),
                    )
                    nc.tensor.matmul(
                        q_ps[:, 512:768], lhsT, wq_sb[:, ck, 512:768],
                        start=(ck == 0), stop=(ck == CK - 1),
                    )
                q_sb = qkvp.tile([128, C], BF16, tag="q")
                nc.scalar.copy(q_sb[:], q_ps[:])
                return q_sb

            def emit_kv(pt, tc_i, v):
                """k,v projection of variant v for one chunk -> SBUF bf16."""
                ts = slice(tc_i * 128, (tc_i + 1) * 128)
                kv_ps = pskv.tile([128, 2 * C], F32, tag="kv")
                for ck in range(CK):
                    lhsT = pt[:, v, ck, ts]
                    for co in range(3):
                        nc.tensor.matmul(
                            kv_ps[:, co * 512 : (co + 1) * 512],
                            lhsT,
                            wkv_sb[:, ck, co * 512 : (co + 1) * 512],
                            start=(ck == 0), stop=(ck == CK - 1),
                        )
                k_sb = qkvp.tile([128, C], BF16, tag=f"k{v}")
                v_sb = qkvp.tile([128, C], BF16, tag=f"v{v}")
                nc.scalar.copy(k_sb[:], kv_ps[:, 0:C])
                nc.scalar.copy(v_sb[:], kv_ps[:, C : 2 * C])
                return k_sb, v_sb

            def emit_logit(q_sb, k_sb, L, v):
                """prod + head-reduce for one variant (DVE); exp on ACT."""
                prod = attp.tile([128, C], BF16, tag=f"prod{v}")
                nc.vector.tensor_mul(prod[:], q_sb[:], k_sb[:])
                nc.vector.tensor_reduce(
                    L[:, v, :],
                    prod[:].rearrange("p (h d) -> p h d", d=D),
                    axis=mybir.AxisListType.X,
                    op=mybir.AluOpType.add,
                )

            def emit_softmax_mix(E, v_sbs):
                """softmax over V + weighted value mix -> att [t, C] bf16."""
                # denominator: sum E over v (strided view, innermost = v)
                ssum = attp.tile([128, 1, H, 1], F32, tag="ssum")
                nc.vector.tensor_reduce(
                    ssum[:, 0, :, 0],
                    E[:].rearrange("p v h -> p h v"),
                    axis=mybir.AxisListType.X,
                    op=mybir.AluOpType.add,
                )
                rcp = attp.tile([128, 1, H, 1], F32, tag="rcp")
                nc.vector.reciprocal(rcp[:], ssum[:])
                # normalized weights, duplicated in adjacent pairs so the
                # d-broadcast multiplies below still read packed bf16 pairs
                W2 = attp.tile([128, V, H, 2], BF16, tag="wgt")
                nc.vector.tensor_mul(
                    W2[:],
                    E[:].unsqueeze(-1).broadcast_to([128, V, H, 2]),
                    rcp[:].broadcast_to([128, V, H, 2]),
                )
                tmp = []
                for v in range(V):
                    tv = attp.tile([128, C], BF16, tag=f"tv{v}")
                    wb = W2[:, v, :, :].unsqueeze(2).broadcast_to([128, H, D // 2, 2])
                    nc.vector.tensor_mul(
                        tv[:].rearrange("p (h e j) -> p h e j", e=D // 2, j=2),
                        v_sbs[v][:].rearrange("p (h e j) -> p h e j", e=D // 2, j=2),
                        wb,
                    )
                    tmp.append(tv)
                a01 = attp.tile([128, C], BF16, tag="a01")
                a23 = attp.tile([128, C], BF16, tag="a23")
                att = attp.tile([128, C], BF16, tag="att")
                nc.vector.tensor_add(a01[:], tmp[0][:], tmp[1][:])
                nc.vector.tensor_add(a23[:], tmp[2][:], tmp[3][:])
                nc.vector.tensor_add(att[:], a01[:], a23[:])
                return att

            def emit_transpose(att):
                attT = attTp.tile([128, CK, 128], BF16, tag="attT")
                for ck in range(CK):
                    nc.sync.dma_start_transpose(
                        attT[:, ck, :], att[:, ck * 128 : (ck + 1) * 128]
                    )
                return attT

            def emit_output(attT, row0):
                """project through Wp, DMA out (bf16; host adds bias)."""
                o_ps = psqo.tile([128, C], F32, tag="qo")
                for ck in range(CK):
                    lhsT = attT[:, ck, :]
                    nc.tensor.matmul(
                        o_ps[:, 0:512], lhsT, wp_sb[:, ck, 0:512],
                        start=(ck == 0), stop=(ck == CK - 1),
                    )
                    nc.tensor.matmul(
                        o_ps[:, 512:768], lhsT, wp_sb[:, ck, 512:768],
                        start=(ck == 0), stop=(ck == CK - 1),
                    )
                o_sb = outp.tile([128, C], BF16, tag="osb")
                nc.scalar.copy(o_sb[:], o_ps[:])
                # store on the ACT HWDGE ring: chains right after the copy on
                # the same engine and keeps the SP ring transpose-only
                nc.scalar.dma_start(out[row0 : row0 + 128, :], o_sb[:])

            def emit_attention(q_sb, xt, pt, tc_i, pending):
                """full per-chunk emission with the kv/logit interleave.

                pending: list of (attT, row0) awaiting output projection;
                out-proj for chunk i-lag is emitted after this chunk's kv0
                group so its PSUM slot (shared with q) is free by then.
                """
                L = attp.tile([128, V, H], F32, tag="logits")
                E = attp.tile([128, V, H], F32, tag="exps")
                k0, v0 = emit_kv(pt, tc_i, 0)
                if pending:
                    emit_output(*pending.pop(0))
                emit_logit(q_sb, k0, L, 0)
                v_sbs = [v0]
                for v in range(1, V):
                    k_sb, v_sb = emit_kv(pt, tc_i, v)
                    v_sbs.append(v_sb)
                    emit_logit(q_sb, k_sb, L, v)
                nc.scalar.activation(E[:], L[:],
                                     mybir.ActivationFunctionType.Exp)
                return emit_softmax_mix(E, v_sbs)

            def emit_body(rep):
                pending = []  # [(attT, row0), ...] with depth `lag`
                xt = pt = None
                for ci in range(n_chunks):
                    it, tc_i = divmod(ci, n_ch)
                    if tc_i == 0:
                        t0 = it * tile_tok
                        if rep == 0 and it == 0:
                            xt, pt = xt0, pt0
                        else:
                            xt = xin.tile([128, CK, tile_tok], BF16, tag="xt")
                            nc.gpsimd.dma_start(
                                xt[:], xT_v[:, :, t0 : t0 + tile_tok]
                            )
                            pt = None
                            if not skip_pt:
                                pt = pin.tile([128, V, CK, tile_tok], BF16, tag="pt")
                                for v in range(V):
                                    nc.gpsimd.dma_start(
                                        pt[:, v], pT_v[:, v, :, t0 : t0 + tile_tok]
                                    )
                    row0 = it * tile_tok + tc_i * 128
                    if (ablate or "").startswith("mmbench"):
                        # pure-PE probe: rq back-to-back q-projection groups,
                        # result DMA'd out directly (no transpose/out-proj)
                        rq = int(ablate[len("mmbench"):] or 1)
                        for _ in range(rq):
                            q_sb = emit_q(xt, tc_i)
                        nc.sync.dma_start(out[row0 : row0 + 128, :], q_sb[:])
                        continue
                    q_sb = emit_q(xt, tc_i)
                    if ablate in ("noattn", "qonly"):
                        att = q_sb
                    elif ablate == "projonly":
                        # keep the full PE stream (q+kv+out) and PSUM->SBUF
                        # copies, but skip the DVE attention chain
                        if pending and len(pending) >= lag:
                            emit_output(*pending.pop(0))
                        for v in range(V):
                            emit_kv(pt, tc_i, v)
                        att = q_sb
                    else:
                        att = emit_attention(q_sb, xt, pt, tc_i, pending
                                             if len(pending) >= lag else [])
                    attT = emit_transpose(att)
                    pending.append((attT, row0))
                    while len(pending) > lag:
                        emit_output(*pending.pop(0))
                for p in pending:
                    emit_output(*p)

            if loop > 1:
                with tc.For_i(0, loop, 1):
                    for _ in range(repeat):
                        emit_body(1)
            else:
                for rep in range(repeat):
                    emit_body(rep)

    _dedupe_ldweights(nc)
    nc.compile()
    # NOTE: _thin_pe_sem_updates measured SLOWER on HW (435us vs 407us):
    # sparse semaphore update events delay waiters; per-MM incs are ~free.
    return nc


def _thin_pe_sem_updates(nc):
    """Drop per-matmul semaphore increments nobody waits on.

    Tile gives every matmul a then_inc on the PE progress semaphore; each
    inc is a serialized EVT_SEM register write (~26ns) between matmuls.
    Consumers only ever wait on a handful of distinct thresholds (group
    finals + sequencer pacing marks), so keep exactly the increments at
    waited-on cumulative positions, remap every wait value to its rank in
    the kept set, and patch the per-iteration reset decrement to the new
    total.  Waits whose position was stripped round UP to the next kept
    increment (fires later -> safe).
    """
    fn = nc.m.functions[0]
    # 1. identify PE progress semaphores: sems updated by PE Matmults
    pe_sems = set()
    for blk in fn.blocks:
        for inst in blk.instructions:
            if isinstance(inst, mybir.InstMatmult) and inst.sync_info:
                for u in inst.sync_info.on_update:
                    if not u.ant_name.startswith("barrier"):
                        pe_sems.add(u.ant_name)
    for sem in pe_sems:
        # 2. collect all wait values on this sem anywhere
        wait_vals = set()
        total_incs = 0
        for blk in fn.blocks:
            for inst in blk.instructions:
                si = inst.sync_info
                if not si:
                    continue
                for w in si.on_wait:
                    if w.ant_name == sem and w.wait_value is not None:
                        wait_vals.add(w.wait_value)
                for u in si.on_update:
                    if u.ant_name == sem and u.update_mode == "sem-inc":
                        total_incs += u.update_value
        kept = sorted(v for v in wait_vals if 0 < v <= total_incs)
        if not kept or total_incs == 0:
            continue
        kept_set = set(kept)
        import bisect
        def new_wait(v):
            if v <= 0:
                return v
            n = bisect.bisect_right(kept, v)
            if v in kept_set:
                return n
            # position stripped: wait for the next kept inc (later -> safe)
            return min(n + 1, len(kept))
        # 3. rewrite: strip non-kept incs, remap waits, patch decrements
        for blk in fn.blocks:
            pos = 0
            for inst in blk.instructions:
                si = inst.sync_info
                if not si:
                    continue
                new_w = list(si.on_wait)
                for w in new_w:
                    if w.ant_name == sem and w.wait_value is not None and w.wait_mode == "sem-ge-imm":
                        w.wait_value = new_wait(w.wait_value)
                new_u = []
                changed = False
                for u in si.on_update:
                    if u.ant_name == sem and u.update_mode == "sem-inc":
                        pos += u.update_value
                        if pos in kept_set:
                            new_u.append(u)
                        else:
                            changed = True
                    elif (u.ant_name == sem
                          and u.update_mode in ("sem-add-imm", "sem-sub-imm", "sem-dec")
                          and u.update_value == total_incs):
                        u.update_value = len(kept)
                        changed = True
                        new_u.append(u)
                    else:
                        new_u.append(u)
                if changed:
                    inst.sync_info = mybir.SyncInfo(on_wait=new_w, on_update=new_u)


def _dedupe_ldweights(nc):
    """Remove back-to-back InstLdweights that reload the identical stationary
    operand the PE already holds.

    Tile lowering emits one Ldweights per matmul, so a group of matmuls that
    share a stationary tile (q/out: 2 per weight block, kv: 3) reloads it
    each time; on HW the ~53ns FWL load is serialized with the matmul
    stream (full-array row groups always conflict, so the reorder window
    cannot pull it ahead).  The PE retains the stationary operand between
    matmuls, so the reloads are pure overhead.  Dependency tracking is
    unaffected: each InstMatmult still carries the weights AP as an operand.
    Any waits/updates on a removed Ldweights migrate to the next PE
    instruction (fires later -> safe).
    """
    fn = nc.m.functions[0]
    removed = 0
    for blk in fn.blocks:
        insts = list(blk.instructions)
        keep = []
        last_key = None
        pending_sync = []  # SyncInfos from removed dup LDWs
        for inst in insts:
            if inst.engine != mybir.EngineType.PE:
                keep.append(inst)
                continue
            if isinstance(inst, mybir.InstLdweights):
                ap = inst.ins[0]
                key = (
                    ap.memref, ap.offset, str(ap.ap), str(ap.dtype),
                    str(inst.perf_mode), str(inst.is_transpose),
                    str(inst.tile_position),
                )
                if key == last_key:
                    si = inst.sync_info
                    if si is not None and (len(si.on_wait) or len(si.on_update)):
                        pending_sync.append(si)
                    removed += 1
                    continue
                last_key = key
                keep.append(inst)
            elif isinstance(inst, mybir.InstMatmult):
                if pending_sync:
                    si = inst.sync_info
                    waits = list(si.on_wait) if si else []
                    upds = list(si.on_update) if si else []
                    for ps in pending_sync:
                        waits.extend(ps.on_wait)
                        upds.extend(ps.on_update)
                    inst.sync_info = mybir.SyncInfo(on_wait=waits, on_update=upds)
                    pending_sync = []
                keep.append(inst)
            else:
                # drains/branches etc: conservatively force a reload after
                last_key = None
                keep.append(inst)
        if removed:
            blk.instructions[:] = keep
    return removed


def _prep_inputs(x, variants_patches, Wq, Wkv, Wp, bp):
    """Host-side: cast to bf16, transpose activations feature-major, shard.

    The attention scale (1/8, exact in binary) is folded into Wq here; bp
    is NOT shipped to the device (added on the host after gathering).
    """
    xs = np.ascontiguousarray(x.reshape(TOK, C).T.astype(nbf16))  # [C, TOK]
    ps = np.ascontiguousarray(
        variants_patches.reshape(V, TOK, C).transpose(0, 2, 1).astype(nbf16)
    )  # [V, C, TOK]
    wq = np.ascontiguousarray((Wq * SCALE).astype(nbf16))
    wkv = np.ascontiguousarray(Wkv.astype(nbf16))
    wp = np.ascontiguousarray(Wp.astype(nbf16))

    in_maps = []
    for c in range(N_CORES):
        sl = slice(c * TPC, (c + 1) * TPC)
        in_maps.append(
            {
                "xT": np.ascontiguousarray(xs[:, sl]),
                "pT": np.ascontiguousarray(ps[:, :, sl]),
                "wq": wq,
                "wkv": wkv,
                "wp": wp,
            }
        )
    return in_maps


_NC_CACHE = {}


def run(x, variants_patches, Wq, Wkv, Wp, bp, **spmd_kwargs):
    if "nc" not in _NC_CACHE:
        _NC_CACHE["nc"] = build_nc()
    nc = _NC_CACHE["nc"]
    in_maps = _prep_inputs(x, variants_patches, Wq, Wkv, Wp, bp)
    res = run_bass_kernel_spmd(nc, in_maps, core_ids=list(range(N_CORES)), **spmd_kwargs)
    full = np.concatenate([res.results[c]["out"] for c in range(N_CORES)], axis=0)
    full = full.astype(np.float32) + bp.reshape(1, C).astype(np.float32)
    return full.reshape(B, N, C), res


def make_runner(nc, in_maps):
    """Compile the SPMD NEFF via the PJRT path; return (run_fn, collect_fn).

    run_fn() executes once (blocking) and returns the raw jax outputs;
    collect_fn(out) converts to per-core result dicts.  Inputs live on
    device; each call re-donates freshly-uploaded zero output buffers.
    """
    import jax
    import time
    from jax.sharding import Mesh, PartitionSpec
    from jax.experimental.shard_map import shard_map
    from concourse import bass2jax, mybir as _mybir
    from concourse.bass2jax import _bass_exec_p, install_neuronx_cc_hook

    install_neuronx_cc_hook()
    n_cores = len(in_maps)
    partition_name = nc.partition_id_tensor.name if nc.partition_id_tensor else None

    in_names, out_names, out_avals, zero_outs = [], [], [], []
    for alloc in nc.m.functions[0].allocations:
        if not isinstance(alloc, _mybir.MemoryLocationSet):
            continue
        name = alloc.memorylocations[0].name
        if alloc.kind == "ExternalInput":
            if name != partition_name:
                in_names.append(name)
        elif alloc.kind == "ExternalOutput":
            shape = tuple(alloc.tensor_shape)
            dtype = _mybir.dt.np(alloc.dtype)
            out_names.append(name)
            out_avals.append(jax.core.ShapedArray(shape, dtype))
            zero_outs.append(np.zeros(shape, dtype))
    n_params = len(in_names)
    n_outs = len(out_avals)
    in_names_all = in_names + out_names
    if partition_name is not None:
        in_names_all.append(partition_name)

    def _body(*args):
        operands = list(args)
        if partition_name is not None:
            operands.append(bass2jax.partition_id_tensor())
        outs = _bass_exec_p.bind(
            *operands,
            out_avals=tuple(out_avals),
            in_names=tuple(in_names_all),
            out_names=tuple(out_names),
            lowering_input_output_aliases=(),
            sim_require_finite=True,
            sim_require_nnan=True,
            nc=nc,
        )
        return tuple(outs)

    devices = jax.devices()[:n_cores]
    mesh = Mesh(np.asarray(devices), ("core",))
    donate = tuple(range(n_params, n_params + n_outs))
    sharded = jax.jit(
        shard_map(
            _body, mesh=mesh,
            in_specs=(PartitionSpec("core"),) * (n_params + n_outs),
            out_specs=(PartitionSpec("core"),) * n_outs,
            check_rep=False,
        ),
        donate_argnums=donate, keep_unused=True,
    )
    sh = jax.sharding.NamedSharding(mesh, PartitionSpec("core"))
    concat_in = [
        jax.device_put(
            np.concatenate([np.asarray(in_maps[c][nm]) for c in range(n_cores)], axis=0),
            sh,
        )
        for nm in in_names
    ]
    def fresh_zeros():
        return [
            jax.device_put(np.zeros((n_cores * z.shape[0], *z.shape[1:]), z.dtype), sh)
            for z in zero_outs
        ]

    def run_fn():
        zs = fresh_zeros()
        jax.block_until_ready(zs)
        t0 = time.perf_counter()
        out = sharded(*concat_in, *zs)
        jax.block_until_ready(out)
        return time.perf_counter() - t0, out

    def collect_fn(out):
        return [
            {nm: np.asarray(out[i]).reshape(n_cores, *out_avals[i].shape)[c]
             for i, nm in enumerate(out_names)}
            for c in range(n_cores)
        ]

    return run_fn, collect_fn


def bench(nc, in_maps, iters=20):
    run_fn, collect_fn = make_runner(nc, in_maps)
    run_fn()  # warmup/compile
    times = []
    out = None
    for _ in range(iters):
        dt, out = run_fn()
        times.append(dt)
    return times, collect_fn(out)


def kernel(x, variants_patches, num_layer=None, Wq=None, Wkv=None, Wp=None, bp=None):
    x = np.asarray(x, dtype=np.float32)
    variants_patches = np.asarray(variants_patches, dtype=np.float32)
    Wq = np.asarray(Wq, dtype=np.float32)
    Wkv = np.asarray(Wkv, dtype=np.float32)
    Wp = np.asarray(Wp, dtype=np.float32)
    bp = np.asarray(bp, dtype=np.float32)
    out, _ = run(x, variants_patches, Wq, Wkv, Wp, bp)
    return out


# revision 20
# speedup vs baseline: 1.0689x; 1.0689x over previous
"""Trainium2 Bass kernel for the variants-attention module.

Model (reference):
    q = (x @ Wq)                          [B,N,H,D]
    kv = variants @ Wkv -> k,v            [V,B,N,H,D] each
    attn = softmax(q.k / sqrt(D)) over V  (per-token attention over variants)
    out = (attn.v) @ Wp + bp              [B,N,C]

Strategy: data-parallel over the B*N = 16384 tokens across 8 NeuronCores
(2048 tokens/core), weights replicated.  Host pre-casts inputs to bf16 and
pre-transposes activations to feature-major so the kernel streams them into
the PE array without on-chip transposes.  All projections run on the tensor
engine in bf16 (fp32 PSUM accumulate).  The attention softmax scale is
folded into Wq on the host (exact: 1/8 is a power of two), and the output
bias bp is added on the host after gathering, so the PE does nothing but
the three projections.  PSUM->SBUF evacuation runs on the scalar (ACT)
engine; the per-token attention over V=4 variants runs on the vector
engine with all tensor_tensor ops in 2x mode (the softmax weights are
written duplicated-in-pairs so the d-broadcast multiply still reads packed
16-bit pairs).  The attended output is transposed back via SBUF->SBUF
xbar DMA-transpose and projected through Wp with a 3-chunk pipeline lag so
the vector-engine chain and transpose DMAs never stall the PE.  Input
tiles stream in on the gpsimd SWDGE ring so the SP HWDGE ring carries
only transposes and output stores.
"""

import numpy as np
import ml_dtypes

import concourse.bass as bass
import concourse.bacc as bacc
import concourse.tile as tile
from concourse import mybir
from concourse.bass_utils import run_bass_kernel_spmd

# ---------------------------------------------------------------------------

V, B, N, C, H = 4, 4, 4096, 768, 12
D = C // H
SCALE = D**-0.5
TOK = B * N
N_CORES = 8
TPC = TOK // N_CORES  # tokens per core

BF16 = mybir.dt.bfloat16
F32 = mybir.dt.float32
CK = C // 128  # 6 feature chunks

nbf16 = ml_dtypes.bfloat16


def build_nc(tpc=TPC, tile_tok=512, repeat=1, loop=1, ablate=None, lag=3):
    """Build the per-core Bass program for `tpc` tokens.

    repeat>1 re-runs the whole computation that many times unrolled;
    loop>1 wraps the body in a hardware For_i loop.  Both are idempotent
    (same outputs) and exist only for timing: with loop~1000 the NEFF's
    execution time dominates the axon dispatch jitter, so wall/loop ~= exec.
    """
    assert tpc % tile_tok == 0 and tile_tok % 128 == 0
    n_tiles = tpc // tile_tok
    n_ch = tile_tok // 128  # 128-token chunks per tile

    nc = bacc.Bacc("TRN2", target_bir_lowering=False, debug=False, num_devices=N_CORES)

    xT = nc.dram_tensor("xT", [C, tpc], BF16, kind="ExternalInput").ap()
    pT = nc.dram_tensor("pT", [V, C, tpc], BF16, kind="ExternalInput").ap()
    wq = nc.dram_tensor("wq", [C, C], BF16, kind="ExternalInput").ap()
    wkv = nc.dram_tensor("wkv", [C, 2 * C], BF16, kind="ExternalInput").ap()
    wp = nc.dram_tensor("wp", [C, C], BF16, kind="ExternalInput").ap()
    out = nc.dram_tensor("out", [tpc, C], BF16, kind="ExternalOutput").ap()

    xT_v = xT.rearrange("(ck p) t -> p ck t", p=128)
    pT_v = pT.rearrange("v (ck p) t -> p v ck t", p=128)

    with tile.TileContext(nc) as tc:
        with (
            tc.tile_pool(name="const", bufs=1) as constp,
            tc.tile_pool(name="xin", bufs=2) as xin,
            tc.tile_pool(name="pin", bufs=2) as pin,
            tc.tile_pool(name="qkv", bufs=2) as qkvp,
            tc.tile_pool(name="attn", bufs=2) as attp,
            tc.tile_pool(name="attT", bufs=4) as attTp,
            tc.tile_pool(name="outs", bufs=2) as outp,
            tc.tile_pool(name="pskv", bufs=2, space="PSUM") as pskv,
            tc.tile_pool(
                name="psqo",
                bufs=4 if (ablate or "").startswith("mmbench") else 1,
                space="PSUM",
            ) as psqo,
        ):
            # --- persistent constants ---
            # first tile's activations load before the big weight tensors so
            # the PE can start as soon as wq + tile0 land.
            xt0 = xin.tile([128, CK, tile_tok], BF16, tag="xt")
            nc.gpsimd.dma_start(xt0[:], xT_v[:, :, 0:tile_tok])
            skip_pt = ablate == "qonly" or (ablate or "").startswith("mmbench")
            pt0 = None
            if not skip_pt:
                pt0 = pin.tile([128, V, CK, tile_tok], BF16, tag="pt")
                for v in range(V):
                    nc.gpsimd.dma_start(pt0[:, v], pT_v[:, v, :, 0:tile_tok])

            wq_sb = constp.tile([128, CK, C], BF16, tag="wq")
            nc.sync.dma_start(wq_sb[:], wq.rearrange("(ck p) o -> p ck o", p=128))
            wkv_sb = constp.tile([128, CK, 2 * C], BF16, tag="wkv")
            nc.sync.dma_start(wkv_sb[:], wkv.rearrange("(ck p) o -> p ck o", p=128))
            wp_sb = constp.tile([128, CK, C], BF16, tag="wp")
            nc.sync.dma_start(wp_sb[:], wp.rearrange("(ck p) o -> p ck o", p=128))

            n_chunks = n_tiles * n_ch

            def emit_q(xt, tc_i):
                """q projection for one 128-token chunk -> SBUF bf16."""
                ts = slice(tc_i * 128, (tc_i + 1) * 128)
                q_ps = psqo.tile([128, C], F32, tag="qo")
                for ck in range(CK):
                    lhsT = xt[:, ck, ts]
                    nc.tensor.matmul(
                        q_ps[:, 0:512], lhsT, wq_sb[:, ck, 0:512],
                        start=(ck == 0), stop=(ck == CK - 1),
                    )
                    nc.tensor.matmul(
                        q_ps[:, 512:768], lhsT, wq_sb[:, ck, 512:768],
                        start=(ck == 0), stop=(ck == CK - 1),
                    )
                q_sb = qkvp.tile([128, C], BF16, tag="q")
                nc.scalar.copy(q_sb[:], q_ps[:])
                return q_sb

            def emit_kv(pt, tc_i, v):
                """k,v projection of variant v for one chunk -> SBUF bf16."""
                ts = slice(tc_i * 128, (tc_i + 1) * 128)
                kv_ps = pskv.tile([128, 2 * C], F32, tag="kv")
                for ck in range(CK):
                    lhsT = pt[:, v, ck, ts]
                    for co in range(3):
                        nc.tensor.matmul(
                            kv_ps[:, co * 512 : (co + 1) * 512],
                            lhsT,
                            wkv_sb[:, ck, co * 512 : (co + 1) * 512],
                            start=(ck == 0), stop=(ck == CK - 1),
                        )
                k_sb = qkvp.tile([128, C], BF16, tag=f"k{v}")
                v_sb = qkvp.tile([128, C], BF16, tag=f"v{v}")
                nc.scalar.copy(k_sb[:], kv_ps[:, 0:C])
                nc.scalar.copy(v_sb[:], kv_ps[:, C : 2 * C])
                return k_sb, v_sb

            def emit_logit(q_sb, k_sb, L, v):
                """prod + head-reduce for one variant (DVE); exp on ACT."""
                prod = attp.tile([128, C], BF16, tag=f"prod{v}")
                nc.vector.tensor_mul(prod[:], q_sb[:], k_sb[:])
                nc.vector.tensor_reduce(
                    L[:, v, :],
                    prod[:].rearrange("p (h d) -> p h d", d=D),
                    axis=mybir.AxisListType.X,
                    op=mybir.AluOpType.add,
                )

            def emit_softmax_mix(E, v_sbs):
                """softmax over V + weighted value mix -> att [t, C] bf16."""
                # denominator: sum E over v (strided view, innermost = v)
                ssum = attp.tile([128, 1, H, 1], F32, tag="ssum")
                nc.vector.tensor_reduce(
                    ssum[:, 0, :, 0],
                    E[:].rearrange("p v h -> p h v"),
                    axis=mybir.AxisListType.X,
                    op=mybir.AluOpType.add,
                )
                rcp = attp.tile([128, 1, H, 1], F32, tag="rcp")
                nc.vector.reciprocal(rcp[:], ssum[:])
                # normalized weights, duplicated in adjacent pairs so the
                # d-broadcast multiplies below still read packed bf16 pairs
                W2 = attp.tile([128, V, H, 2], BF16, tag="wgt")
                nc.vector.tensor_mul(
                    W2[:],
                    E[:].unsqueeze(-1).broadcast_to([128, V, H, 2]),
                    rcp[:].broadcast_to([128, V, H, 2]),
                )
                tmp = []
                for v in range(V):
                    tv = attp.tile([128, C], BF16, tag=f"tv{v}")
                    wb = W2[:, v, :, :].unsqueeze(2).broadcast_to([128, H, D // 2, 2])
                    nc.vector.tensor_mul(
                        tv[:].rearrange("p (h e j) -> p h e j", e=D // 2, j=2),
                        v_sbs[v][:].rearrange("p (h e j) -> p h e j", e=D // 2, j=2),
                        wb,
                    )
                    tmp.append(tv)
                a01 = attp.tile([128, C], BF16, tag="a01")
                a23 = attp.tile([128, C], BF16, tag="a23")
                att = attp.tile([128, C], BF16, tag="att")
                nc.vector.tensor_add(a01[:], tmp[0][:], tmp[1][:])
                nc.vector.tensor_add(a23[:], tmp[2][:], tmp[3][:])
                nc.vector.tensor_add(att[:], a01[:], a23[:])
                return att

            def emit_transpose(att):
                attT = attTp.tile([128, CK, 128], BF16, tag="attT")
                for ck in range(CK):
                    nc.sync.dma_start_transpose(
                        attT[:, ck, :], att[:, ck * 128 : (ck + 1) * 128]
                    )
                return attT

            def emit_output(attT, row0):
                """project through Wp, DMA out (bf16; host adds bias)."""
                o_ps = psqo.tile([128, C], F32, tag="qo")
                for ck in range(CK):
                    lhsT = attT[:, ck, :]
                    nc.tensor.matmul(
                        o_ps[:, 0:512], lhsT, wp_sb[:, ck, 0:512],
                        start=(ck == 0), stop=(ck == CK - 1),
                    )
                    nc.tensor.matmul(
                        o_ps[:, 512:768], lhsT, wp_sb[:, ck, 512:768],
                        start=(ck == 0), stop=(ck == CK - 1),
                    )
                o_sb = outp.tile([128, C], BF16, tag="osb")
                nc.scalar.copy(o_sb[:], o_ps[:])
                nc.sync.dma_start(out[row0 : row0 + 128, :], o_sb[:])

            def emit_attention(q_sb, xt, pt, tc_i, pending):
                """full per-chunk emission with the kv/logit interleave.

                pending: list of (attT, row0) awaiting output projection;
                out-proj for chunk i-lag is emitted after this chunk's kv0
                group so its PSUM slot (shared with q) is free by then.
                """
                L = attp.tile([128, V, H], F32, tag="logits")
                E = attp.tile([128, V, H], F32, tag="exps")
                k0, v0 = emit_kv(pt, tc_i, 0)
                if pending:
                    emit_output(*pending.pop(0))
                emit_logit(q_sb, k0, L, 0)
                v_sbs = [v0]
                for v in range(1, V):
                    k_sb, v_sb = emit_kv(pt, tc_i, v)
                    v_sbs.append(v_sb)
                    emit_logit(q_sb, k_sb, L, v)
                nc.scalar.activation(E[:], L[:],
                                     mybir.ActivationFunctionType.Exp)
                return emit_softmax_mix(E, v_sbs)

            def emit_body(rep):
                pending = []  # [(attT, row0), ...] with depth `lag`
                xt = pt = None
                for ci in range(n_chunks):
                    it, tc_i = divmod(ci, n_ch)
                    if tc_i == 0:
                        t0 = it * tile_tok
                        if rep == 0 and it == 0:
                            xt, pt = xt0, pt0
                        else:
                            xt = xin.tile([128, CK, tile_tok], BF16, tag="xt")
                            nc.gpsimd.dma_start(
                                xt[:], xT_v[:, :, t0 : t0 + tile_tok]
                            )
                            pt = None
                            if not skip_pt:
                                pt = pin.tile([128, V, CK, tile_tok], BF16, tag="pt")
                                for v in range(V):
                                    nc.gpsimd.dma_start(
                                        pt[:, v], pT_v[:, v, :, t0 : t0 + tile_tok]
                                    )
                    row0 = it * tile_tok + tc_i * 128
                    if (ablate or "").startswith("mmbench"):
                        # pure-PE probe: rq back-to-back q-projection groups,
                        # result DMA'd out directly (no transpose/out-proj)
                        rq = int(ablate[len("mmbench"):] or 1)
                        for _ in range(rq):
                            q_sb = emit_q(xt, tc_i)
                        nc.sync.dma_start(out[row0 : row0 + 128, :], q_sb[:])
                        continue
                    q_sb = emit_q(xt, tc_i)
                    if ablate in ("noattn", "qonly"):
                        att = q_sb
                    elif ablate == "projonly":
                        # keep the full PE stream (q+kv+out) and PSUM->SBUF
                        # copies, but skip the DVE attention chain
                        if pending and len(pending) >= lag:
                            emit_output(*pending.pop(0))
                        for v in range(V):
                            emit_kv(pt, tc_i, v)
                        att = q_sb
                    else:
                        att = emit_attention(q_sb, xt, pt, tc_i, pending
                                             if len(pending) >= lag else [])
                    attT = emit_transpose(att)
                    pending.append((attT, row0))
                    while len(pending) > lag:
                        emit_output(*pending.pop(0))
                for p in pending:
                    emit_output(*p)

            if loop > 1:
                with tc.For_i(0, loop, 1):
                    for _ in range(repeat):
                        emit_body(1)
            else:
                for rep in range(repeat):
                    emit_body(rep)

    _dedupe_ldweights(nc)
    nc.compile()
    # NOTE: _thin_pe_sem_updates measured SLOWER on HW (435us vs 407us):
    # sparse semaphore update events delay waiters; per-MM incs are ~free.
    return nc


def _thin_pe_sem_updates(nc):
    """Drop per-matmul semaphore increments nobody waits on.

    Tile gives every matmul a then_inc on the PE progress semaphore; each
    inc is a serialized EVT_SEM register write (~26ns) between matmuls.
    Consumers only ever wait on a handful of distinct thresholds (group
    finals + sequencer pacing marks), so keep exactly the increments at
    waited-on cumulative positions, remap every wait value to its rank in
    the kept set, and patch the per-iteration reset decrement to the new
    total.  Waits whose position was stripped round UP to the next kept
    increment (fires later -> safe).
    """
    fn = nc.m.functions[0]
    # 1. identify PE progress semaphores: sems updated by PE Matmults
    pe_sems = set()
    for blk in fn.blocks:
        for inst in blk.instructions:
            if isinstance(inst, mybir.InstMatmult) and inst.sync_info:
                for u in inst.sync_info.on_update:
                    if not u.ant_name.startswith("barrier"):
                        pe_sems.add(u.ant_name)
    for sem in pe_sems:
        # 2. collect all wait values on this sem anywhere
        wait_vals = set()
        total_incs = 0
        for blk in fn.blocks:
            for inst in blk.instructions:
                si = inst.sync_info
                if not si:
                    continue
                for w in si.on_wait:
                    if w.ant_name == sem and w.wait_value is not None:
                        wait_vals.add(w.wait_value)
                for u in si.on_update:
                    if u.ant_name == sem and u.update_mode == "sem-inc":
                        total_incs += u.update_value
        kept = sorted(v for v in wait_vals if 0 < v <= total_incs)
        if not kept or total_incs == 0:
            continue
        kept_set = set(kept)
        import bisect
        def new_wait(v):
            if v <= 0:
                return v
            n = bisect.bisect_right(kept, v)
            if v in kept_set:
                return n
            # position stripped: wait for the next kept inc (later -> safe)
            return min(n + 1, len(kept))
        # 3. rewrite: strip non-kept incs, remap waits, patch decrements
        for blk in fn.blocks:
            pos = 0
            for inst in blk.instructions:
                si = inst.sync_info
                if not si:
                    continue
                new_w = list(si.on_wait)
                for w in new_w:
                    if w.ant_name == sem and w.wait_value is not None and w.wait_mode == "sem-ge-imm":
                        w.wait_value = new_wait(w.wait_value)
                new_u = []
                changed = False
                for u in si.on_update:
                    if u.ant_name == sem and u.update_mode == "sem-inc":
                        pos += u.update_value
                        if pos in kept_set:
                            new_u.append(u)
                        else:
                            changed = True
                    elif (u.ant_name == sem
                          and u.update_mode in ("sem-add-imm", "sem-sub-imm", "sem-dec")
                          and u.update_value == total_incs):
                        u.update_value = len(kept)
                        changed = True
                        new_u.append(u)
                    else:
                        new_u.append(u)
                if changed:
                    inst.sync_info = mybir.SyncInfo(on_wait=new_w, on_update=new_u)


def _dedupe_ldweights(nc):
    """Remove back-to-back InstLdweights that reload the identical stationary
    operand the PE already holds.

    Tile lowering emits one Ldweights per matmul, so a group of matmuls that
    share a stationary tile (q/out: 2 per weight block, kv: 3) reloads it
    each time; on HW the ~53ns FWL load is serialized with the matmul
    stream (full-array row groups always conflict, so the reorder window
    cannot pull it ahead).  The PE retains the stationary operand between
    matmuls, so the reloads are pure overhead.  Dependency tracking is
    unaffected: each InstMatmult still carries the weights AP as an operand.
    Any waits/updates on a removed Ldweights migrate to the next PE
    instruction (fires later -> safe).
    """
    fn = nc.m.functions[0]
    removed = 0
    for blk in fn.blocks:
        insts = list(blk.instructions)
        keep = []
        last_key = None
        pending_sync = []  # SyncInfos from removed dup LDWs
        for inst in insts:
            if inst.engine != mybir.EngineType.PE:
                keep.append(inst)
                continue
            if isinstance(inst, mybir.InstLdweights):
                ap = inst.ins[0]
                key = (
                    ap.memref, ap.offset, str(ap.ap), str(ap.dtype),
                    str(inst.perf_mode), str(inst.is_transpose),
                    str(inst.tile_position),
                )
                if key == last_key:
                    si = inst.sync_info
                    if si is not None and (len(si.on_wait) or len(si.on_update)):
                        pending_sync.append(si)
                    removed += 1
                    continue
                last_key = key
                keep.append(inst)
            elif isinstance(inst, mybir.InstMatmult):
                if pending_sync:
                    si = inst.sync_info
                    waits = list(si.on_wait) if si else []
                    upds = list(si.on_update) if si else []
                    for ps in pending_sync:
                        waits.extend(ps.on_wait)
                        upds.extend(ps.on_update)
                    inst.sync_info = mybir.SyncInfo(on_wait=waits, on_update=upds)
                    pending_sync = []
                keep.append(inst)
            else:
                # drains/branches etc: conservatively force a reload after
                last_key = None
                keep.append(inst)
        if removed:
            blk.instructions[:] = keep
    return removed


def _prep_inputs(x, variants_patches, Wq, Wkv, Wp, bp):
    """Host-side: cast to bf16, transpose activations feature-major, shard.

    The attention scale (1/8, exact in binary) is folded into Wq here; bp
    is NOT shipped to the device (added on the host after gathering).
    """
    xs = np.ascontiguousarray(x.reshape(TOK, C).T.astype(nbf16))  # [C, TOK]
    ps = np.ascontiguousarray(
        variants_patches.reshape(V, TOK, C).transpose(0, 2, 1).astype(nbf16)
    )  # [V, C, TOK]
    wq = np.ascontiguousarray((Wq * SCALE).astype(nbf16))
    wkv = np.ascontiguousarray(Wkv.astype(nbf16))
    wp = np.ascontiguousarray(Wp.astype(nbf16))

    in_maps = []
    for c in range(N_CORES):
        sl = slice(c * TPC, (c + 1) * TPC)
        in_maps.append(
            {
                "xT": np.ascontiguousarray(xs[:, sl]),
                "pT": np.ascontiguousarray(ps[:, :, sl]),
                "wq": wq,
                "wkv": wkv,
                "wp": wp,
            }
        )
    return in_maps


_NC_CACHE = {}


def run(x, variants_patches, Wq, Wkv, Wp, bp, **spmd_kwargs):
    if "nc" not in _NC_CACHE:
        _NC_CACHE["nc"] = build_nc()
    nc = _NC_CACHE["nc"]
    in_maps = _prep_inputs(x, variants_patches, Wq, Wkv, Wp, bp)
    res = run_bass_kernel_spmd(nc, in_maps, core_ids=list(range(N_CORES)), **spmd_kwargs)
    full = np.concatenate([res.results[c]["out"] for c in range(N_CORES)], axis=0)
    full = full.astype(np.float32) + bp.reshape(1, C).astype(np.float32)
    return full.reshape(B, N, C), res


def make_runner(nc, in_maps):
    """Compile the SPMD NEFF via the PJRT path; return (run_fn, collect_fn).

    run_fn() executes once (blocking) and returns the raw jax outputs;
    collect_fn(out) converts to per-core result dicts.  Inputs live on
    device; each call re-donates freshly-uploaded zero output buffers.
    """
    import jax
    import time
    from jax.sharding import Mesh, PartitionSpec
    from jax.experimental.shard_map import shard_map
    from concourse import bass2jax, mybir as _mybir
    from concourse.bass2jax import _bass_exec_p, install_neuronx_cc_hook

    install_neuronx_cc_hook()
    n_cores = len(in_maps)
    partition_name = nc.partition_id_tensor.name if nc.partition_id_tensor else None

    in_names, out_names, out_avals, zero_outs = [], [], [], []
    for alloc in nc.m.functions[0].allocations:
        if not isinstance(alloc, _mybir.MemoryLocationSet):
            continue
        name = alloc.memorylocations[0].name
        if alloc.kind == "ExternalInput":
            if name != partition_name:
                in_names.append(name)
        elif alloc.kind == "ExternalOutput":
            shape = tuple(alloc.tensor_shape)
            dtype = _mybir.dt.np(alloc.dtype)
            out_names.append(name)
            out_avals.append(jax.core.ShapedArray(shape, dtype))
            zero_outs.append(np.zeros(shape, dtype))
    n_params = len(in_names)
    n_outs = len(out_avals)
    in_names_all = in_names + out_names
    if partition_name is not None:
        in_names_all.append(partition_name)

    def _body(*args):
        operands = list(args)
        if partition_name is not None:
            operands.append(bass2jax.partition_id_tensor())
        outs = _bass_exec_p.bind(
            *operands,
            out_avals=tuple(out_avals),
            in_names=tuple(in_names_all),
            out_names=tuple(out_names),
            lowering_input_output_aliases=(),
            sim_require_finite=True,
            sim_require_nnan=True,
            nc=nc,
        )
        return tuple(outs)

    devices = jax.devices()[:n_cores]
    mesh = Mesh(np.asarray(devices), ("core",))
    donate = tuple(range(n_params, n_params + n_outs))
    sharded = jax.jit(
        shard_map(
            _body, mesh=mesh,
            in_specs=(PartitionSpec("core"),) * (n_params + n_outs),
            out_specs=(PartitionSpec("core"),) * n_outs,
            check_rep=False,
        ),
        donate_argnums=donate, keep_unused=True,
    )
    sh = jax.sharding.NamedSharding(mesh, PartitionSpec("core"))
    concat_in = [
        jax.device_put(
            np.concatenate([np.asarray(in_maps[c][nm]) for c in range(n_cores)], axis=0),
            sh,
        )
        for nm in in_names
    ]
    def fresh_zeros():
        return [
            jax.device_put(np.zeros((n_cores * z.shape[0], *z.shape[1:]), z.dtype), sh)
            for z in zero_outs
        ]

    def run_fn():
        zs = fresh_zeros()
        jax.block_until_ready(zs)
        t0 = time.perf_counter()
        out = sharded(*concat_in, *zs)
        jax.block_until_ready(out)
        return time.perf_counter() - t0, out

    def collect_fn(out):
        return [
            {nm: np.asarray(out[i]).reshape(n_cores, *out_avals[i].shape)[c]
             for i, nm in enumerate(out_names)}
            for c in range(n_cores)
        ]

    return run_fn, collect_fn


def bench(nc, in_maps, iters=20):
    run_fn, collect_fn = make_runner(nc, in_maps)
    run_fn()  # warmup/compile
    times = []
    out = None
    for _ in range(iters):
        dt, out = run_fn()
        times.append(dt)
    return times, collect_fn(out)


def kernel(x, variants_patches, num_layer=None, Wq=None, Wkv=None, Wp=None, bp=None):
    x = np.asarray(x, dtype=np.float32)
    variants_patches = np.asarray(variants_patches, dtype=np.float32)
    Wq = np.asarray(Wq, dtype=np.float32)
    Wkv = np.asarray(Wkv, dtype=np.float32)
    Wp = np.asarray(Wp, dtype=np.float32)
    bp = np.asarray(bp, dtype=np.float32)
    out, _ = run(x, variants_patches, Wq, Wkv, Wp, bp)
    return out
